# revision 39
# baseline (speedup 1.0000x reference)
# MultiLoraConv2d kernel for 8 trn2 NeuronCores (Bass/Tile, data-parallel over batch).
#
# Math (per sample b):
#   delta_flat[b] = sum_t 2*alphas[b,t] * (lora_B[t] @ lora_A[t])        [768, 768]
#   agg[b] = W + delta_flat[b].reshape(COUT, CIN, 3, 3)                  (flat reinterpret)
#   out[b] = conv2d(x[b], agg[b], pad=1)
#
# Device strategy (per core, S = B/8 samples):
#   - All matmul operands bf16 (1 cycle/row on PE, same as fp32r; halves DMA
#     + SBUF). PSUM accumulation stays fp32; max rel err ~2.3e-3 (gate 2e-2).
#   - Host pre-lays-out tensors partition-first; LoRA factors regrouped by
#     (d = 3*i + j, s = (c*9+d)//768) so per-sample aggregated conv weights
#     come out of the PE directly in c-major (stationary) layout:
#       S_d[c, o] = sum_s sum_r a3[d,s,r,c] * (2*alpha_{t(r)} * b3[s,r,o])
#   - Conv = 18 shifted matmuls (9 taps x 2 cin tiles) per PSUM bank;
#     measured issue cadence ~218 ns per 512-col bf16 matmul (hw floor).
#   - Phase 1 does 2 matmuls per bank, not 3: s=(c*9+d)//768 spans only two
#     of the three flat-reshape thirds within any 128-wide c block, so one
#     stationary is all-zero (108 -> 72 matmuls).
#   - Phase-1 eviction (dp + W -> ws2) must keep up with 436ns/bank matmul
#     production: a direct DVE add from PSUM is 691ns (fp32 PSUM operand
#     blocks DVE's 2x mode), so banks are produced into 3-bank PSUM group
#     tiles, Act fuses each group's PSUM->bf16 copy into one ACTIVATE
#     (~1.5us/group), and DVE adds W from the bf16 tmp at the 2x rate
#     (~417ns/bank). GpSimd is useless here: its tensor_scalar is ~3.9us
#     and adds ~1.4us (sw-implemented), and it cannot touch PSUM.
#   - Schedule: 12 dummy warmup matmuls ramp the PE p-state during the ~8us
#     NEFF boot + ~11us first-DMA latency window (wz zeroed on GpSimd so
#     warmup starts right at Tensor boot, not after Act's table load); DMAs
#     in consumption order (alph + full b3 first, a3 chunks, wt interleaved
#     with x prefetches); b3s muls split DVE/Act in consumption order; conv
#     PSUM->SBUF copies alternate DVE/Act; single 8-bank PSUM pool (2x3-bank
#     dp3 groups + 2 pb ring slots, warmup folded into the dp3 tag); output
#     batched 1 DMA per (smp, ot), last sample split per-hb and the final
#     bank split into two 4-row halves to shrink the drain tail.
import numpy as np

B, T, R, ALPHA = 32, 4, 8, 16
CIN, COUT, K = 256, 256, 3
H = W_SP = 64
SCALING = ALPHA / R
NCORES = 8
S = B // NCORES      # samples per core
NR = T * R * K       # 96 lora rows (padded to 128 partitions)
P = 128
HP = H + 2           # 66 padded

_CACHE = {}


def _build_nc():
    import concourse.bacc as bacc
    import concourse.mybir as mybir
    import concourse.tile as tile

    f32 = mybir.dt.float32
    bf16 = mybir.dt.bfloat16

    nc = bacc.Bacc("TRN2", target_bir_lowering=False, debug=False, num_devices=NCORES)

    xp = nc.declare_dram_parameter("xp", [S, 2, P, HP, HP], bf16, isOutput=False)
    wt = nc.declare_dram_parameter("wt", [P, 9, 2, 2, COUT], bf16, isOutput=False)
    a3 = nc.declare_dram_parameter("a3", [NR, 9, 3, CIN], bf16, isOutput=False)
    b3 = nc.declare_dram_parameter("b3", [NR, 3, COUT], bf16, isOutput=False)
    alph = nc.declare_dram_parameter("alph", [NR, S], f32, isOutput=False)
    outd = nc.declare_dram_parameter("out", [S, 2, P, H, W_SP], f32, isOutput=True)

    with tile.TileContext(nc) as tc:
        with tc.tile_pool(name="persist", bufs=1) as persist, \
             tc.tile_pool(name="xt_pool", bufs=6) as xt_pool, \
             tc.tile_pool(name="ob_pool", bufs=2) as ob_pool, \
             tc.tile_pool(name="ps", bufs=1, space="PSUM") as ps:
            a3_sb = persist.tile([P, 9, 3, CIN], bf16)
            b3_sb = persist.tile([P, 3, COUT], bf16)
            alph_sb = persist.tile([P, S], f32)
            wt_sb = persist.tile([P, 9, 2, 2, COUT], bf16)
            b3s = [persist.tile([P, 3, 2, COUT], bf16, name=f"b3s{h}")
                   for h in range(2)]
            ws2 = [persist.tile([P, 9, S, COUT], bf16, name=f"ws2{c}")
                   for c in range(2)]
            wz = persist.tile([P, 704], bf16)
            tmp3 = [persist.tile([P, 3, 2, COUT], bf16, name=f"tmp3{g}")
                    for g in range(2)]

            # PE warmup: p-state ramps to full clock after ~5us of continuous
            # execution; burn that in during NEFF boot / first DMAs so real
            # matmuls run near 2.4GHz from the start (9 was measured too few:
            # the first ~10 real matmuls ran at 427ns). wz is zeroed on
            # GpSimd (boots at ~0.1us) so warmup starts right at Tensor
            # engine boot (~7.2us) instead of waiting for Act's 1.3us
            # ACT_TABLE_LOAD + memzero (~9.5us). The dummy Act copy still
            # pulls the ACT_TABLE_LOAD off the b3s-mul critical path (its
            # target range is outside the warmup APs so it gates nothing).
            nc.gpsimd.memset(wz[:, :], 0.0)
            nc.scalar.copy(wz[:, 672:688], wz[:, 640:656])
            warm = ps.tile([P, 3, 2, COUT], f32, name="dp3", bufs=2)
            for _ in range(12):
                nc.tensor.matmul(warm[:, 0, :, :], wz[:, 0:128], wz[:, 128:640],
                                 start=True, stop=True)

            # DMAs in consumption order on the SP queue (issuing from a
            # second DGE queue halves per-queue DMA bandwidth kernel-wide —
            # measured +60us). alph + full b3 first (gate the b3s muls),
            # a3 next (gates phase-1 matmuls), wt interleaved with the x0
            # prefetches; first DMA completion lands ~11.3us (pipe boot).
            nc.sync.dma_start(alph_sb[0:NR, :], alph[:, :])
            nc.sync.dma_start(b3_sb[0:NR, :], b3[:, :])
            nc.sync.dma_start(a3_sb[0:NR, 0:3], a3[:, 0:3])
            nc.sync.dma_start(a3_sb[0:NR, 3:6], a3[:, 3:6])
            nc.sync.dma_start(a3_sb[0:NR, 6:9], a3[:, 6:9])
            # (wt is host-doubled along a j=2 dim so the phase-1 W-add is a
            # single [P,2,256] tensor_add per (h,d,ct) with no broadcast AP)
            xts = [[None, None] for _ in range(S)]
            nc.sync.dma_start(wt_sb[:, 0:3], wt[:, 0:3])
            xts[0][0] = xt_pool.tile([P, HP, HP], bf16, name="xt")
            nc.sync.dma_start(xts[0][0][:, :, :], xp[0, 0, :, :, :])
            nc.sync.dma_start(wt_sb[:, 3:6], wt[:, 3:6])
            xts[0][1] = xt_pool.tile([P, HP, HP], bf16, name="xt")
            nc.sync.dma_start(xts[0][1][:, :, :], xp[0, 1, :, :, :])
            nc.sync.dma_start(wt_sb[:, 6:9], wt[:, 6:9])
            for smp in range(1, 3):
                for ct in range(2):
                    t = xt_pool.tile([P, HP, HP], bf16, name="xt")
                    nc.sync.dma_start(t[:, :, :], xp[smp, ct, :, :, :])
                    xts[smp][ct] = t

            # b3s[h][:, s, j, :] = (2*alphas[2h+j]) * b3[s]  (bf16 out; the
            # 2x scaling is folded into the host-side alph values). DVE
            # (278ns/mul) takes all of h0 plus h1's s0 in consumption
            # order; Act (584ns/mul) takes h1's s1/s2, not needed until
            # bank 19 (~8us later). Moving Act's muls to DVE was tried:
            # it un-delays Act's first ring-freeing copy but overloads DVE
            # (muls + all W-adds) and the ring stall just moves there.
            for s, j in ((0, 0), (0, 1), (1, 0), (1, 1), (2, 0), (2, 1)):
                nc.vector.tensor_scalar_mul(b3s[0][0:NR, s, j, :],
                                            b3_sb[0:NR, s, :],
                                            alph_sb[0:NR, j:j + 1])
            for s, j in ((0, 0), (0, 1)):
                nc.vector.tensor_scalar_mul(b3s[1][0:NR, s, j, :],
                                            b3_sb[0:NR, s, :],
                                            alph_sb[0:NR, 2 + j:3 + j])
            for s, j in ((1, 0), (1, 1), (2, 0), (2, 1)):
                nc.scalar.mul(b3s[1][0:NR, s, j, :], b3_sb[0:NR, s, :],
                              alph_sb[0:NR, 2 + j:3 + j])

            # ---- phase 1: aggregated weights via LoRA matmuls ----
            # dp[c, j, o] = sum_s sum_r a3[d,s,r,c] * b3s[h][s,j,o]; then
            # ws2[ct][c, d, 2h+j, o] = dp[c, j, o] + W[c, d, ct, o].
            # s = (c*9+d)//768, so a 128-wide c block spans only TWO of the
            # three s thirds: ct=0 -> s in {0,1}, ct=1 -> s in {1,2}; the
            # third matmul's stationary is all-zero — skip it (108 -> 72).
            # Eviction must keep up with 436ns/bank matmul production.
            # Measured costs: DVE direct add from PSUM 691ns (fp32 PSUM
            # operand blocks the 2x mode); Act fused 3-bank PSUM->bf16 copy
            # 1541ns (513/bank); DVE all-bf16 2x add 417ns; GpSimd all-SBUF
            # add ~1.4us (sw-implemented; its tensor_scalar is 3.9us — do
            # NOT put muls there). No two engines alone cover the 36 banks
            # inside the 15.7us matmul window, so spread over three:
            # 5 groups direct DVE adds, 4 groups Act-copied + DVE adds,
            # 3 groups Act-copied + GpSimd adds -> DVE ~15.4us, Act ~10.8,
            # GpSimd ~12.4. GpSimd takes the LAST h1 groups: it never holds
            # a PSUM slot (the Act copy frees it) and its banks are not
            # read until conv smp2 (~90us), so even 2x slowness is safe.
            ebank = [(h, d, ct)
                     for h in range(2) for d in range(9) for ct in range(2)]

            def bank_matmuls(out_ap, h, d, ct):
                s_pair = (0, 1) if ct == 0 else (1, 2)
                for k, s in enumerate(s_pair):
                    nc.tensor.matmul(
                        out_ap,
                        a3_sb[0:NR, d, s, ct * P:(ct + 1) * P],
                        b3s[h][0:NR, s, :, :],
                        start=(k == 0), stop=(k == 1))

            def direct_evict(sb1, h, d, ct):
                nc.vector.tensor_add(ws2[ct][:, d, 2 * h:2 * h + 2, :],
                                     sb1[:, :, :], wt_sb[:, d, ct, :, :])

            # Banks 0, 1 and 35 run through single-bank tiles borrowed from
            # the pb pool (idle until conv ~30us) with direct DVE adds.
            # This grants the startup exactly two extra PSUM ring slots, so
            # Act's first fused group-copy completes before the 2-deep dp3
            # ring wraps (the wrap cost ~1.5us x3 + a p-state droop).
            for e in range(2):
                h, d, ct = ebank[e]
                sb1 = ps.tile([P, 2, COUT], f32, name="pb", bufs=2)
                bank_matmuls(sb1[:, :, :], h, d, ct)
                direct_evict(sb1, h, d, ct)
            for g in range(11):
                dpz = ps.tile([P, 3, 2, COUT], f32, name="dp3", bufs=2)
                for i in range(3):
                    h, d, ct = ebank[2 + 3 * g + i]
                    bank_matmuls(dpz[:, i, :, :], h, d, ct)
                t3 = tmp3[g % 2]
                nc.scalar.copy(t3[:, :, :, :], dpz[:, :, :, :])
                for i in range(3):
                    h, d, ct = ebank[2 + 3 * g + i]
                    nc.vector.tensor_add(
                        ws2[ct][:, d, 2 * h:2 * h + 2, :],
                        t3[:, i, :, :], wt_sb[:, d, ct, :, :])
            h, d, ct = ebank[35]
            sb1 = ps.tile([P, 2, COUT], f32, name="pb", bufs=2)
            bank_matmuls(sb1[:, :, :], h, d, ct)
            direct_evict(sb1, h, d, ct)

            # ---- phase 2: per-sample conv, 18 shifted matmuls per psum bank ----
            for smp in range(S):
                if smp == 1:
                    for ct in range(2):
                        t = xt_pool.tile([P, HP, HP], bf16, name="xt")
                        nc.sync.dma_start(t[:, :, :], xp[3, ct, :, :, :])
                        xts[3][ct] = t
                for ot in range(2):
                    ob = ob_pool.tile([P, H, W_SP], f32, name="ob")
                    last = (smp == S - 1)
                    for hb in range(8):
                        if last and ot == 1 and hb == 7:
                            # Final bank split 4+2+2 rows so the drain after
                            # the very last matmul is a 2-row copy + 64KB
                            # DMA; earlier sub-banks' copy/DMA overlap the
                            # later sub-banks' matmuls (~0.1us extra PE
                            # issue overhead, measured -0.36us drain).
                            for y0, rows, eng in ((56, 4, nc.vector),
                                                  (60, 2, nc.scalar),
                                                  (62, 2, nc.vector)):
                                pb = ps.tile([P, 8, W_SP], f32, name="pb",
                                             bufs=2)
                                first = True
                                for ct in range(2):
                                    for d in range(9):
                                        di, dj = divmod(d, 3)
                                        nc.tensor.matmul(
                                            pb[:, 0:rows, :],
                                            ws2[ct][:, d, smp,
                                                    ot * P:(ot + 1) * P],
                                            xts[smp][ct][:, y0 + di:
                                                         y0 + di + rows,
                                                         dj:dj + W_SP],
                                            start=first,
                                            stop=(ct == 1 and d == 8))
                                        first = False
                                if eng is nc.vector:
                                    nc.vector.tensor_copy(
                                        ob[:, y0:y0 + rows, :],
                                        pb[:, 0:rows, :])
                                else:
                                    nc.scalar.copy(
                                        ob[:, y0:y0 + rows, :],
                                        pb[:, 0:rows, :])
                                nc.sync.dma_start(
                                    outd[smp, ot, :, y0:y0 + rows, :],
                                    ob[:, y0:y0 + rows, :])
                            continue
                        pb = ps.tile([P, 8, W_SP], f32, name="pb", bufs=2)
                        first = True
                        for ct in range(2):
                            for d in range(9):
                                di, dj = divmod(d, 3)
                                loc = hb * 8 + di
                                nc.tensor.matmul(
                                    pb[:, :, :],
                                    ws2[ct][:, d, smp, ot * P:(ot + 1) * P],
                                    xts[smp][ct][:, loc:loc + 8, dj:dj + W_SP],
                                    start=first, stop=(ct == 1 and d == 8))
                                first = False
                        if hb % 2 == 0:
                            nc.vector.tensor_copy(
                                ob[:, hb * 8:(hb + 1) * 8, :], pb[:, :, :])
                        else:
                            nc.scalar.copy(
                                ob[:, hb * 8:(hb + 1) * 8, :], pb[:, :, :])
                        if last and ot == 1 and hb >= 4:
                            nc.sync.dma_start(
                                outd[smp, ot, :, hb * 8:(hb + 1) * 8, :],
                                ob[:, hb * 8:(hb + 1) * 8, :])
                        elif last and hb % 2 == 1:
                            k = hb // 2
                            nc.sync.dma_start(
                                outd[smp, ot, :, k * 16:(k + 1) * 16, :],
                                ob[:, k * 16:(k + 1) * 16, :])
                    if not last:
                        nc.sync.dma_start(outd[smp, ot, :, :, :], ob[:, :, :])
    nc.finalize()
    return nc


def _host_prep(x, alphas, W, lora_A, lora_B):
    """Host-side layout/dtype transforms (pad/transpose/gather/cast)."""
    import ml_dtypes
    bf16 = ml_dtypes.bfloat16

    xf = np.asarray(x, dtype=np.float32)
    af = np.asarray(alphas, dtype=np.float32)
    Wf = np.asarray(W, dtype=np.float32)
    Af = np.asarray(lora_A, dtype=np.float32).reshape(NR, CIN * K)   # Acat
    Bf = np.asarray(lora_B, dtype=np.float32)

    # padded x, per core: (S, 2, 128, 66, 66) bf16
    xpad = np.zeros((B, CIN, HP, HP), bf16)
    xpad[:, :, 1:-1, 1:-1] = xf.astype(bf16)
    xpad = xpad.reshape(NCORES, S, 2, P, HP, HP)

    # base weights c-major, d-major free layout, doubled along j so the
    # device-side eviction add needs no broadcast: wt[p, d, ct, j, o]
    wth = np.ascontiguousarray(
        Wf.reshape(COUT, CIN, 9).transpose(1, 2, 0)        # [c, d, o]
        .reshape(2, P, 9, COUT)                            # [ct, p, d, o]
        .transpose(1, 2, 0, 3)).astype(bf16)               # [p, d, ct, o]
    wth = np.ascontiguousarray(
        np.broadcast_to(wth[:, :, :, None, :], (P, 9, 2, 2, COUT)))

    # a3[r, d, s, c] = Acat[r, c*9+d-768*s] masked; rows padded 96 -> 128
    a3h = np.zeros((P, 9, 3, CIN), np.float32)
    cc = np.arange(CIN)
    for d in range(9):
        q = cc * 9 + d
        s_of_c = q // (CIN * K)
        q_of_c = q % (CIN * K)
        for s in range(3):
            m = s_of_c == s
            a3h[:NR, d, s, m] = Af[:, q_of_c[m]]
    a3h = a3h[:NR].astype(bf16)

    # b3[r, s, o] = Bcat[3o+s, r];  Bcat = lora_B transposed to [768, 96]
    Bcat = Bf.transpose(1, 0, 2).reshape(COUT * K, NR)
    b3h = np.zeros((P, 3, COUT), np.float32)
    b3h[:NR] = Bcat.reshape(COUT, 3, NR).transpose(2, 1, 0)
    b3h = b3h[:NR].astype(bf16)

    # alph[r, smp] per core (repeat each task 24x; zero rows >= 96).
    # SCALING (alpha/r = 2) folded in here so the device skips one mul.
    alphh = np.zeros((NCORES, NR, S), np.float32)
    rep = np.repeat(af * SCALING, R * K, axis=1)           # [B, 96]
    alphh[:, :, :] = rep.reshape(NCORES, S, NR).transpose(0, 2, 1)

    return xpad, wth, a3h, b3h, alphh


def _in_maps(x, alphas, W, lora_A, lora_B):
    xpad, wth, a3h, b3h, alphh = _host_prep(x, alphas, W, lora_A, lora_B)
    return [
        {"xp": np.ascontiguousarray(xpad[c]), "wt": wth, "a3": a3h, "b3": b3h,
         "alph": np.ascontiguousarray(alphh[c])}
        for c in range(NCORES)
    ]


def kernel(x, alphas, W, lora_A, lora_B):
    from concourse.bass_utils import run_bass_kernel_spmd

    if "nc" not in _CACHE:
        _CACHE["nc"] = _build_nc()
    nc = _CACHE["nc"]

    in_maps = _in_maps(x, alphas, W, lora_A, lora_B)
    res = run_bass_kernel_spmd(nc, in_maps, list(range(NCORES)))
    out = np.empty((B, COUT, H, W_SP), np.float32)
    for c in range(NCORES):
        out[c * S:(c + 1) * S] = res.results[c]["out"].reshape(S, COUT, H, W_SP)
    return out



# revision 41
# speedup vs baseline: 1.0005x; 1.0005x over previous
# MultiLoraConv2d kernel for 8 trn2 NeuronCores (Bass/Tile, data-parallel over batch).
#
# Math (per sample b):
#   delta_flat[b] = sum_t 2*alphas[b,t] * (lora_B[t] @ lora_A[t])        [768, 768]
#   agg[b] = W + delta_flat[b].reshape(COUT, CIN, 3, 3)                  (flat reinterpret)
#   out[b] = conv2d(x[b], agg[b], pad=1)
#
# Device strategy (per core, S = B/8 samples):
#   - All matmul operands bf16 (1 cycle/row on PE, same as fp32r; halves DMA
#     + SBUF). PSUM accumulation stays fp32; max rel err ~2.3e-3 (gate 2e-2).
#   - Host pre-lays-out tensors partition-first; LoRA factors regrouped by
#     (d = 3*i + j, s = (c*9+d)//768) so per-sample aggregated conv weights
#     come out of the PE directly in c-major (stationary) layout:
#       S_d[c, o] = sum_s sum_r a3[d,s,r,c] * (2*alpha_{t(r)} * b3[s,r,o])
#   - Conv = 18 shifted matmuls (9 taps x 2 cin tiles) per PSUM bank;
#     measured issue cadence ~218 ns per 512-col bf16 matmul (hw floor).
#   - Phase 1 does 2 matmuls per bank, not 3: s=(c*9+d)//768 spans only two
#     of the three flat-reshape thirds within any 128-wide c block, so one
#     stationary is all-zero (108 -> 72 matmuls).
#   - Phase-1 eviction (dp + W -> ws2) must keep up with 436ns/bank matmul
#     production: a direct DVE add from PSUM is 691ns (fp32 PSUM operand
#     blocks DVE's 2x mode), so banks are produced into 3-bank PSUM group
#     tiles, Act fuses each group's PSUM->bf16 copy into one ACTIVATE
#     (~1.5us/group), and DVE adds W from the bf16 tmp at the 2x rate
#     (~417ns/bank). GpSimd is useless here: its tensor_scalar is ~3.9us
#     and adds ~1.4us (sw-implemented), and it cannot touch PSUM.
#   - Schedule: 12 dummy warmup matmuls ramp the PE p-state during the ~8us
#     NEFF boot + ~11us first-DMA latency window (wz zeroed on GpSimd so
#     warmup starts right at Tensor boot, not after Act's table load); DMAs
#     in consumption order (alph + full b3 first, a3 chunks, wt interleaved
#     with x prefetches); b3s muls split DVE/Act in consumption order; conv
#     PSUM->SBUF copies alternate DVE/Act; single 8-bank PSUM pool (2x3-bank
#     dp3 groups + 2 pb ring slots, warmup folded into the dp3 tag); output
#     batched 1 DMA per (smp, ot), last sample split per-hb and the final
#     bank split into two 4-row halves to shrink the drain tail.
import numpy as np

B, T, R, ALPHA = 32, 4, 8, 16
CIN, COUT, K = 256, 256, 3
H = W_SP = 64
SCALING = ALPHA / R
NCORES = 8
S = B // NCORES      # samples per core
NR = T * R * K       # 96 lora rows (padded to 128 partitions)
P = 128
HP = H + 2           # 66 padded

_CACHE = {}


def _build_nc():
    import concourse.bacc as bacc
    import concourse.mybir as mybir
    import concourse.tile as tile

    f32 = mybir.dt.float32
    bf16 = mybir.dt.bfloat16

    nc = bacc.Bacc("TRN2", target_bir_lowering=False, debug=False, num_devices=NCORES)

    xp = nc.declare_dram_parameter("xp", [S, 2, P, HP, HP], bf16, isOutput=False)
    wt = nc.declare_dram_parameter("wt", [P, 9, 2, 2, COUT], bf16, isOutput=False)
    a3 = nc.declare_dram_parameter("a3", [NR, 9, 3, CIN], bf16, isOutput=False)
    b3 = nc.declare_dram_parameter("b3", [NR, 3, COUT], bf16, isOutput=False)
    alph = nc.declare_dram_parameter("alph", [NR, S], f32, isOutput=False)
    outd = nc.declare_dram_parameter("out", [S, 2, P, H, W_SP], f32, isOutput=True)

    with tile.TileContext(nc) as tc:
        with tc.tile_pool(name="persist", bufs=1) as persist, \
             tc.tile_pool(name="xt_pool", bufs=6) as xt_pool, \
             tc.tile_pool(name="ob_pool", bufs=2) as ob_pool, \
             tc.tile_pool(name="ps", bufs=1, space="PSUM") as ps:
            a3_sb = persist.tile([P, 9, 3, CIN], bf16)
            b3_sb = persist.tile([P, 3, COUT], bf16)
            alph_sb = persist.tile([P, S], f32)
            wt_sb = persist.tile([P, 9, 2, 2, COUT], bf16)
            b3s = [persist.tile([P, 3, 2, COUT], bf16, name=f"b3s{h}")
                   for h in range(2)]
            ws2 = [persist.tile([P, 9, S, COUT], bf16, name=f"ws2{c}")
                   for c in range(2)]
            wz = persist.tile([P, 704], bf16)
            tmp3 = [persist.tile([P, 3, 2, COUT], bf16, name=f"tmp3{g}")
                    for g in range(2)]

            # PE warmup: p-state ramps to full clock after ~5us of continuous
            # execution; burn that in during NEFF boot / first DMAs so real
            # matmuls run near 2.4GHz from the start (9 was measured too few:
            # the first ~10 real matmuls ran at 427ns). wz is zeroed on
            # GpSimd (boots at ~0.1us) so warmup starts right at Tensor
            # engine boot (~7.2us) instead of waiting for Act's 1.3us
            # ACT_TABLE_LOAD + memzero (~9.5us). The dummy Act copy still
            # pulls the ACT_TABLE_LOAD off the b3s-mul critical path (its
            # target range is outside the warmup APs so it gates nothing).
            nc.gpsimd.memset(wz[:, :], 0.0)
            nc.scalar.copy(wz[:, 672:688], wz[:, 640:656])
            warm = ps.tile([P, 3, 2, COUT], f32, name="dp3", bufs=2)
            for _ in range(12):
                nc.tensor.matmul(warm[:, 0, :, :], wz[:, 0:128], wz[:, 128:640],
                                 start=True, stop=True)

            # DMAs in consumption order on the SP queue (issuing from a
            # second DGE queue halves per-queue DMA bandwidth kernel-wide —
            # measured +60us). alph + full b3 first (gate the b3s muls),
            # a3 next (gates phase-1 matmuls), wt interleaved with the x0
            # prefetches; first DMA completion lands ~11.3us (pipe boot).
            nc.sync.dma_start(alph_sb[0:NR, :], alph[:, :])
            nc.sync.dma_start(b3_sb[0:NR, :], b3[:, :])
            nc.sync.dma_start(a3_sb[0:NR, 0:3], a3[:, 0:3])
            nc.sync.dma_start(a3_sb[0:NR, 3:6], a3[:, 3:6])
            nc.sync.dma_start(a3_sb[0:NR, 6:9], a3[:, 6:9])
            # (wt is host-doubled along a j=2 dim so the phase-1 W-add is a
            # single [P,2,256] tensor_add per (h,d,ct) with no broadcast AP)
            xts = [[None, None] for _ in range(S)]
            nc.sync.dma_start(wt_sb[:, 0:3], wt[:, 0:3])
            xts[0][0] = xt_pool.tile([P, HP, HP], bf16, name="xt")
            nc.sync.dma_start(xts[0][0][:, :, :], xp[0, 0, :, :, :])
            nc.sync.dma_start(wt_sb[:, 3:6], wt[:, 3:6])
            xts[0][1] = xt_pool.tile([P, HP, HP], bf16, name="xt")
            nc.sync.dma_start(xts[0][1][:, :, :], xp[0, 1, :, :, :])
            nc.sync.dma_start(wt_sb[:, 6:9], wt[:, 6:9])
            for smp in range(1, 3):
                for ct in range(2):
                    t = xt_pool.tile([P, HP, HP], bf16, name="xt")
                    nc.sync.dma_start(t[:, :, :], xp[smp, ct, :, :, :])
                    xts[smp][ct] = t

            # b3s[h][:, s, j, :] = (2*alphas[2h+j]) * b3[s]  (bf16 out; the
            # 2x scaling is folded into the host-side alph values). DVE
            # (278ns/mul) takes all of h0 plus h1's s0 in consumption
            # order; Act (584ns/mul) takes h1's s1/s2, not needed until
            # bank 19 (~8us later). Moving Act's muls to DVE was tried:
            # it un-delays Act's first ring-freeing copy but overloads DVE
            # (muls + all W-adds) and the ring stall just moves there.
            for s, j in ((0, 0), (0, 1), (1, 0), (1, 1), (2, 0), (2, 1)):
                nc.vector.tensor_scalar_mul(b3s[0][0:NR, s, j, :],
                                            b3_sb[0:NR, s, :],
                                            alph_sb[0:NR, j:j + 1])
            for s, j in ((0, 0), (0, 1)):
                nc.vector.tensor_scalar_mul(b3s[1][0:NR, s, j, :],
                                            b3_sb[0:NR, s, :],
                                            alph_sb[0:NR, 2 + j:3 + j])
            for s, j in ((1, 0), (1, 1), (2, 0), (2, 1)):
                nc.scalar.mul(b3s[1][0:NR, s, j, :], b3_sb[0:NR, s, :],
                              alph_sb[0:NR, 2 + j:3 + j])

            # ---- phase 1: aggregated weights via LoRA matmuls ----
            # dp[c, j, o] = sum_s sum_r a3[d,s,r,c] * b3s[h][s,j,o]; then
            # ws2[ct][c, d, 2h+j, o] = dp[c, j, o] + W[c, d, ct, o].
            # s = (c*9+d)//768, so a 128-wide c block spans only TWO of the
            # three s thirds: ct=0 -> s in {0,1}, ct=1 -> s in {1,2}; the
            # third matmul's stationary is all-zero — skip it (108 -> 72).
            # Eviction must keep up with 436ns/bank matmul production.
            # Measured costs: DVE direct add from PSUM 691ns (fp32 PSUM
            # operand blocks the 2x mode); Act fused 3-bank PSUM->bf16 copy
            # 1541ns (513/bank); DVE all-bf16 2x add 417ns; GpSimd all-SBUF
            # add ~1.4us (sw-implemented; its tensor_scalar is 3.9us — do
            # NOT put muls there). No two engines alone cover the 36 banks
            # inside the 15.7us matmul window, so spread over three:
            # 5 groups direct DVE adds, 4 groups Act-copied + DVE adds,
            # 3 groups Act-copied + GpSimd adds -> DVE ~15.4us, Act ~10.8,
            # GpSimd ~12.4. GpSimd takes the LAST h1 groups: it never holds
            # a PSUM slot (the Act copy frees it) and its banks are not
            # read until conv smp2 (~90us), so even 2x slowness is safe.
            ebank = [(h, d, ct)
                     for h in range(2) for d in range(9) for ct in range(2)]
            for g in range(12):
                dpz = ps.tile([P, 3, 2, COUT], f32, name="dp3", bufs=2)
                for i in range(3):
                    h, d, ct = ebank[3 * g + i]
                    s_pair = (0, 1) if ct == 0 else (1, 2)
                    for k, s in enumerate(s_pair):
                        nc.tensor.matmul(
                            dpz[:, i, :, :],
                            a3_sb[0:NR, d, s, ct * P:(ct + 1) * P],
                            b3s[h][0:NR, s, :, :],
                            start=(k == 0), stop=(k == 1))
                t3 = tmp3[g % 2]
                nc.scalar.copy(t3[:, :, :, :], dpz[:, :, :, :])
                for i in range(3):
                    h, d, ct = ebank[3 * g + i]
                    nc.vector.tensor_add(
                        ws2[ct][:, d, 2 * h:2 * h + 2, :],
                        t3[:, i, :, :], wt_sb[:, d, ct, :, :])

            # ---- phase 2: per-sample conv, 18 shifted matmuls per psum bank ----
            for smp in range(S):
                if smp == 1:
                    for ct in range(2):
                        t = xt_pool.tile([P, HP, HP], bf16, name="xt")
                        nc.sync.dma_start(t[:, :, :], xp[3, ct, :, :, :])
                        xts[3][ct] = t
                for ot in range(2):
                    ob = ob_pool.tile([P, H, W_SP], f32, name="ob")
                    last = (smp == S - 1)
                    for hb in range(8):
                        if last and ot == 1 and hb == 7:
                            # Final bank split 4+2+2 rows so the drain after
                            # the very last matmul is a 2-row copy + 64KB
                            # DMA; earlier sub-banks' copy/DMA overlap the
                            # later sub-banks' matmuls (~0.1us extra PE
                            # issue overhead, measured -0.36us drain).
                            for y0, rows, eng in ((56, 4, nc.vector),
                                                  (60, 2, nc.scalar),
                                                  (62, 2, nc.vector)):
                                pb = ps.tile([P, 8, W_SP], f32, name="pb",
                                             bufs=2)
                                first = True
                                for ct in range(2):
                                    for d in range(9):
                                        di, dj = divmod(d, 3)
                                        nc.tensor.matmul(
                                            pb[:, 0:rows, :],
                                            ws2[ct][:, d, smp,
                                                    ot * P:(ot + 1) * P],
                                            xts[smp][ct][:, y0 + di:
                                                         y0 + di + rows,
                                                         dj:dj + W_SP],
                                            start=first,
                                            stop=(ct == 1 and d == 8))
                                        first = False
                                if eng is nc.vector:
                                    nc.vector.tensor_copy(
                                        ob[:, y0:y0 + rows, :],
                                        pb[:, 0:rows, :])
                                else:
                                    nc.scalar.copy(
                                        ob[:, y0:y0 + rows, :],
                                        pb[:, 0:rows, :])
                                nc.sync.dma_start(
                                    outd[smp, ot, :, y0:y0 + rows, :],
                                    ob[:, y0:y0 + rows, :])
                            continue
                        pb = ps.tile([P, 8, W_SP], f32, name="pb", bufs=2)
                        first = True
                        for ct in range(2):
                            for d in range(9):
                                di, dj = divmod(d, 3)
                                loc = hb * 8 + di
                                nc.tensor.matmul(
                                    pb[:, :, :],
                                    ws2[ct][:, d, smp, ot * P:(ot + 1) * P],
                                    xts[smp][ct][:, loc:loc + 8, dj:dj + W_SP],
                                    start=first, stop=(ct == 1 and d == 8))
                                first = False
                        if hb % 2 == 0:
                            nc.vector.tensor_copy(
                                ob[:, hb * 8:(hb + 1) * 8, :], pb[:, :, :])
                        else:
                            nc.scalar.copy(
                                ob[:, hb * 8:(hb + 1) * 8, :], pb[:, :, :])
                        if last and ot == 1 and hb >= 4:
                            nc.sync.dma_start(
                                outd[smp, ot, :, hb * 8:(hb + 1) * 8, :],
                                ob[:, hb * 8:(hb + 1) * 8, :])
                        elif last and hb % 2 == 1:
                            k = hb // 2
                            nc.sync.dma_start(
                                outd[smp, ot, :, k * 16:(k + 1) * 16, :],
                                ob[:, k * 16:(k + 1) * 16, :])
                    if not last:
                        nc.sync.dma_start(outd[smp, ot, :, :, :], ob[:, :, :])
    nc.finalize()
    return nc


def _host_prep(x, alphas, W, lora_A, lora_B):
    """Host-side layout/dtype transforms (pad/transpose/gather/cast)."""
    import ml_dtypes
    bf16 = ml_dtypes.bfloat16

    xf = np.asarray(x, dtype=np.float32)
    af = np.asarray(alphas, dtype=np.float32)
    Wf = np.asarray(W, dtype=np.float32)
    Af = np.asarray(lora_A, dtype=np.float32).reshape(NR, CIN * K)   # Acat
    Bf = np.asarray(lora_B, dtype=np.float32)

    # padded x, per core: (S, 2, 128, 66, 66) bf16
    xpad = np.zeros((B, CIN, HP, HP), bf16)
    xpad[:, :, 1:-1, 1:-1] = xf.astype(bf16)
    xpad = xpad.reshape(NCORES, S, 2, P, HP, HP)

    # base weights c-major, d-major free layout, doubled along j so the
    # device-side eviction add needs no broadcast: wt[p, d, ct, j, o]
    wth = np.ascontiguousarray(
        Wf.reshape(COUT, CIN, 9).transpose(1, 2, 0)        # [c, d, o]
        .reshape(2, P, 9, COUT)                            # [ct, p, d, o]
        .transpose(1, 2, 0, 3)).astype(bf16)               # [p, d, ct, o]
    wth = np.ascontiguousarray(
        np.broadcast_to(wth[:, :, :, None, :], (P, 9, 2, 2, COUT)))

    # a3[r, d, s, c] = Acat[r, c*9+d-768*s] masked; rows padded 96 -> 128
    a3h = np.zeros((P, 9, 3, CIN), np.float32)
    cc = np.arange(CIN)
    for d in range(9):
        q = cc * 9 + d
        s_of_c = q // (CIN * K)
        q_of_c = q % (CIN * K)
        for s in range(3):
            m = s_of_c == s
            a3h[:NR, d, s, m] = Af[:, q_of_c[m]]
    a3h = a3h[:NR].astype(bf16)

    # b3[r, s, o] = Bcat[3o+s, r];  Bcat = lora_B transposed to [768, 96]
    Bcat = Bf.transpose(1, 0, 2).reshape(COUT * K, NR)
    b3h = np.zeros((P, 3, COUT), np.float32)
    b3h[:NR] = Bcat.reshape(COUT, 3, NR).transpose(2, 1, 0)
    b3h = b3h[:NR].astype(bf16)

    # alph[r, smp] per core (repeat each task 24x; zero rows >= 96).
    # SCALING (alpha/r = 2) folded in here so the device skips one mul.
    alphh = np.zeros((NCORES, NR, S), np.float32)
    rep = np.repeat(af * SCALING, R * K, axis=1)           # [B, 96]
    alphh[:, :, :] = rep.reshape(NCORES, S, NR).transpose(0, 2, 1)

    return xpad, wth, a3h, b3h, alphh


def _in_maps(x, alphas, W, lora_A, lora_B):
    xpad, wth, a3h, b3h, alphh = _host_prep(x, alphas, W, lora_A, lora_B)
    return [
        {"xp": np.ascontiguousarray(xpad[c]), "wt": wth, "a3": a3h, "b3": b3h,
         "alph": np.ascontiguousarray(alphh[c])}
        for c in range(NCORES)
    ]


def kernel(x, alphas, W, lora_A, lora_B):
    from concourse.bass_utils import run_bass_kernel_spmd

    if "nc" not in _CACHE:
        _CACHE["nc"] = _build_nc()
    nc = _CACHE["nc"]

    in_maps = _in_maps(x, alphas, W, lora_A, lora_B)
    res = run_bass_kernel_spmd(nc, in_maps, list(range(NCORES)))
    out = np.empty((B, COUT, H, W_SP), np.float32)
    for c in range(NCORES):
        out[c * S:(c + 1) * S] = res.results[c]["out"].reshape(S, COUT, H, W_SP)
    return out



# revision 43
# speedup vs baseline: 1.0065x; 1.0060x over previous
# MultiLoraConv2d kernel for 8 trn2 NeuronCores (Bass/Tile, data-parallel over batch).
#
# Math (per sample b):
#   delta_flat[b] = sum_t 2*alphas[b,t] * (lora_B[t] @ lora_A[t])        [768, 768]
#   agg[b] = W + delta_flat[b].reshape(COUT, CIN, 3, 3)                  (flat reinterpret)
#   out[b] = conv2d(x[b], agg[b], pad=1)
#
# Device strategy (per core, S = B/8 samples):
#   - All matmul operands bf16 (1 cycle/row on PE, same as fp32r; halves DMA
#     + SBUF). PSUM accumulation stays fp32; max rel err ~2.3e-3 (gate 2e-2).
#   - Host pre-lays-out tensors partition-first; LoRA factors regrouped by
#     (d = 3*i + j, s = (c*9+d)//768) so per-sample aggregated conv weights
#     come out of the PE directly in c-major (stationary) layout:
#       S_d[c, o] = sum_s sum_r a3[d,s,r,c] * (2*alpha_{t(r)} * b3[s,r,o])
#   - Conv = 18 shifted matmuls (9 taps x 2 cin tiles) per PSUM bank;
#     measured issue cadence ~218 ns per 512-col bf16 matmul (hw floor).
#   - Phase 1 does 2 matmuls per bank, not 3: s=(c*9+d)//768 spans only two
#     of the three flat-reshape thirds within any 128-wide c block, so one
#     stationary is all-zero (108 -> 72 matmuls).
#   - Phase-1 eviction (dp + W -> ws2) must keep up with 436ns/bank matmul
#     production: a direct DVE add from PSUM is 691ns (fp32 PSUM operand
#     blocks DVE's 2x mode), so banks are produced into 3-bank PSUM group
#     tiles, Act fuses each group's PSUM->bf16 copy into one ACTIVATE
#     (~1.5us/group), and DVE adds W from the bf16 tmp at the 2x rate
#     (~417ns/bank). GpSimd is useless here: its tensor_scalar is ~3.9us
#     and adds ~1.4us (sw-implemented), and it cannot touch PSUM.
#   - Schedule: 12 dummy warmup matmuls ramp the PE p-state during the ~8us
#     NEFF boot + ~11us first-DMA latency window (wz zeroed on GpSimd so
#     warmup starts right at Tensor boot, not after Act's table load); DMAs
#     in consumption order (alph + full b3 first, a3 chunks, wt interleaved
#     with x prefetches); b3s muls split DVE/Act in consumption order; conv
#     PSUM->SBUF copies alternate DVE/Act; single 8-bank PSUM pool (2x3-bank
#     dp3 groups + 2 pb ring slots, warmup folded into the dp3 tag); output
#     batched 1 DMA per (smp, ot), last sample split per-hb and the final
#     bank split into two 4-row halves to shrink the drain tail.
import numpy as np

B, T, R, ALPHA = 32, 4, 8, 16
CIN, COUT, K = 256, 256, 3
H = W_SP = 64
SCALING = ALPHA / R
NCORES = 8
S = B // NCORES      # samples per core
NR = T * R * K       # 96 lora rows (padded to 128 partitions)
P = 128
HP = H + 2           # 66 padded

_CACHE = {}


def _build_nc():
    import concourse.bacc as bacc
    import concourse.mybir as mybir
    import concourse.tile as tile

    f32 = mybir.dt.float32
    bf16 = mybir.dt.bfloat16

    nc = bacc.Bacc("TRN2", target_bir_lowering=False, debug=False, num_devices=NCORES)

    xp = nc.declare_dram_parameter("xp", [S, 2, P, HP, HP], bf16, isOutput=False)
    wt = nc.declare_dram_parameter("wt", [P, 9, 2, 2, COUT], bf16, isOutput=False)
    a3 = nc.declare_dram_parameter("a3", [NR, 9, 3, CIN], bf16, isOutput=False)
    b3 = nc.declare_dram_parameter("b3", [NR, 3, COUT], bf16, isOutput=False)
    alph = nc.declare_dram_parameter("alph", [NR, S], f32, isOutput=False)
    outd = nc.declare_dram_parameter("out", [S, 2, P, H, W_SP], f32, isOutput=True)

    with tile.TileContext(nc) as tc:
        with tc.tile_pool(name="persist", bufs=1) as persist, \
             tc.tile_pool(name="xt_pool", bufs=6) as xt_pool, \
             tc.tile_pool(name="ob_pool", bufs=2) as ob_pool, \
             tc.tile_pool(name="ps", bufs=1, space="PSUM") as ps:
            a3_sb = persist.tile([P, 9, 3, CIN], bf16)
            b3_sb = persist.tile([P, 3, COUT], bf16)
            alph_sb = persist.tile([P, S], f32)
            wt_sb = persist.tile([P, 9, 2, 2, COUT], bf16)
            b3s = [persist.tile([P, 3, 2, COUT], bf16, name=f"b3s{h}")
                   for h in range(2)]
            ws2 = [persist.tile([P, 9, S, COUT], bf16, name=f"ws2{c}")
                   for c in range(2)]
            wz = persist.tile([P, 704], bf16)
            tmp3 = [persist.tile([P, 3, 2, COUT], bf16, name=f"tmp3{g}")
                    for g in range(4)]

            # PE warmup: p-state ramps to full clock after ~5us of continuous
            # execution; burn that in during NEFF boot / first DMAs so real
            # matmuls run near 2.4GHz from the start (9 was measured too few:
            # the first ~10 real matmuls ran at 427ns). wz is zeroed on
            # GpSimd (boots at ~0.1us) so warmup starts right at Tensor
            # engine boot (~7.2us) instead of waiting for Act's 1.3us
            # ACT_TABLE_LOAD + memzero (~9.5us). The dummy Act copy still
            # pulls the ACT_TABLE_LOAD off the b3s-mul critical path (its
            # target range is outside the warmup APs so it gates nothing).
            nc.gpsimd.memset(wz[:, :], 0.0)
            nc.scalar.copy(wz[:, 672:688], wz[:, 640:656])
            warm = ps.tile([P, 3, 2, COUT], f32, name="dp3", bufs=2)
            for _ in range(12):
                nc.tensor.matmul(warm[:, 0, :, :], wz[:, 0:128], wz[:, 128:640],
                                 start=True, stop=True)

            # DMAs in consumption order on the SP queue (issuing from a
            # second DGE queue halves per-queue DMA bandwidth kernel-wide —
            # measured +60us). alph + full b3 first (gate the b3s muls),
            # a3 next (gates phase-1 matmuls), wt interleaved with the x0
            # prefetches; first DMA completion lands ~11.3us (pipe boot).
            nc.sync.dma_start(alph_sb[0:NR, :], alph[:, :])
            nc.sync.dma_start(b3_sb[0:NR, :], b3[:, :])
            nc.sync.dma_start(a3_sb[0:NR, 0:3], a3[:, 0:3])
            nc.sync.dma_start(a3_sb[0:NR, 3:6], a3[:, 3:6])
            nc.sync.dma_start(a3_sb[0:NR, 6:9], a3[:, 6:9])
            # (wt is host-doubled along a j=2 dim so the phase-1 W-add is a
            # single [P,2,256] tensor_add per (h,d,ct) with no broadcast AP)
            xts = [[None, None] for _ in range(S)]
            nc.sync.dma_start(wt_sb[:, 0:3], wt[:, 0:3])
            xts[0][0] = xt_pool.tile([P, HP, HP], bf16, name="xt")
            nc.sync.dma_start(xts[0][0][:, :, :], xp[0, 0, :, :, :])
            nc.sync.dma_start(wt_sb[:, 3:6], wt[:, 3:6])
            xts[0][1] = xt_pool.tile([P, HP, HP], bf16, name="xt")
            nc.sync.dma_start(xts[0][1][:, :, :], xp[0, 1, :, :, :])
            nc.sync.dma_start(wt_sb[:, 6:9], wt[:, 6:9])
            for smp in range(1, 3):
                for ct in range(2):
                    t = xt_pool.tile([P, HP, HP], bf16, name="xt")
                    nc.sync.dma_start(t[:, :, :], xp[smp, ct, :, :, :])
                    xts[smp][ct] = t

            # b3s[h][:, s, j, :] = (2*alphas[2h+j]) * b3[s]  (bf16 out; the
            # 2x scaling is folded into the host-side alph values). DVE
            # (278ns/mul) takes all of h0 plus h1's s0 in consumption
            # order; Act (584ns/mul) takes h1's s1/s2, not needed until
            # bank 19 (~8us later). Moving Act's muls to DVE was tried:
            # it un-delays Act's first ring-freeing copy but overloads DVE
            # (muls + all W-adds) and the ring stall just moves there.
            for s, j in ((0, 0), (0, 1), (1, 0), (1, 1), (2, 0), (2, 1)):
                nc.vector.tensor_scalar_mul(b3s[0][0:NR, s, j, :],
                                            b3_sb[0:NR, s, :],
                                            alph_sb[0:NR, j:j + 1])
            for s, j in ((0, 0), (0, 1)):
                nc.vector.tensor_scalar_mul(b3s[1][0:NR, s, j, :],
                                            b3_sb[0:NR, s, :],
                                            alph_sb[0:NR, 2 + j:3 + j])
            for s, j in ((1, 0), (1, 1), (2, 0), (2, 1)):
                nc.scalar.mul(b3s[1][0:NR, s, j, :], b3_sb[0:NR, s, :],
                              alph_sb[0:NR, 2 + j:3 + j])

            # ---- phase 1: aggregated weights via LoRA matmuls ----
            # dp[c, j, o] = sum_s sum_r a3[d,s,r,c] * b3s[h][s,j,o]; then
            # ws2[ct][c, d, 2h+j, o] = dp[c, j, o] + W[c, d, ct, o].
            # s = (c*9+d)//768, so a 128-wide c block spans only TWO of the
            # three s thirds: ct=0 -> s in {0,1}, ct=1 -> s in {1,2}; the
            # third matmul's stationary is all-zero — skip it (108 -> 72).
            # Eviction must keep up with 436ns/bank matmul production.
            # Measured costs: DVE direct add from PSUM 691ns (fp32 PSUM
            # operand blocks the 2x mode); Act fused 3-bank PSUM->bf16 copy
            # 1541ns (513/bank); DVE all-bf16 2x add 417ns; GpSimd all-SBUF
            # add ~1.4us (sw-implemented; its tensor_scalar is 3.9us — do
            # NOT put muls there). No two engines alone cover the 36 banks
            # inside the 15.7us matmul window, so spread over three:
            # 5 groups direct DVE adds, 4 groups Act-copied + DVE adds,
            # 3 groups Act-copied + GpSimd adds -> DVE ~15.4us, Act ~10.8,
            # GpSimd ~12.4. GpSimd takes the LAST h1 groups: it never holds
            # a PSUM slot (the Act copy frees it) and its banks are not
            # read until conv smp2 (~90us), so even 2x slowness is safe.
            ebank = [(h, d, ct)
                     for h in range(2) for d in range(9) for ct in range(2)]
            for g in range(12):
                dpz = ps.tile([P, 3, 2, COUT], f32, name="dp3", bufs=2)
                for i in range(3):
                    h, d, ct = ebank[3 * g + i]
                    s_pair = (0, 1) if ct == 0 else (1, 2)
                    for k, s in enumerate(s_pair):
                        nc.tensor.matmul(
                            dpz[:, i, :, :],
                            a3_sb[0:NR, d, s, ct * P:(ct + 1) * P],
                            b3s[h][0:NR, s, :, :],
                            start=(k == 0), stop=(k == 1))
                # PSUM-free pacing: all-Act copies free banks at 513ns/bank
                # vs 436ns/bank matmul production (ring-wrap stalls, ~3us);
                # moving every 4th group's copy to DVE (~1.8us fused, run in
                # DVE's idle gaps) brings both engines under production
                # rate. tmp ring of 4 keeps the lagging W-adds from gating
                # the copies via WAR reuse.
                t3 = tmp3[g % 4]
                if g % 4 == 3:
                    nc.vector.tensor_copy(t3[:, :, :, :], dpz[:, :, :, :])
                else:
                    nc.scalar.copy(t3[:, :, :, :], dpz[:, :, :, :])
                for i in range(3):
                    h, d, ct = ebank[3 * g + i]
                    nc.vector.tensor_add(
                        ws2[ct][:, d, 2 * h:2 * h + 2, :],
                        t3[:, i, :, :], wt_sb[:, d, ct, :, :])

            # ---- phase 2: per-sample conv, 18 shifted matmuls per psum bank ----
            for smp in range(S):
                if smp == 1:
                    for ct in range(2):
                        t = xt_pool.tile([P, HP, HP], bf16, name="xt")
                        nc.sync.dma_start(t[:, :, :], xp[3, ct, :, :, :])
                        xts[3][ct] = t
                for ot in range(2):
                    ob = ob_pool.tile([P, H, W_SP], f32, name="ob")
                    last = (smp == S - 1)
                    for hb in range(8):
                        if last and ot == 1 and hb == 7:
                            # Final bank split 4+2+2 rows so the drain after
                            # the very last matmul is a 2-row copy + 64KB
                            # DMA; earlier sub-banks' copy/DMA overlap the
                            # later sub-banks' matmuls (~0.1us extra PE
                            # issue overhead, measured -0.36us drain).
                            for y0, rows, eng in ((56, 4, nc.vector),
                                                  (60, 2, nc.scalar),
                                                  (62, 2, nc.vector)):
                                pb = ps.tile([P, 8, W_SP], f32, name="pb",
                                             bufs=2)
                                first = True
                                for ct in range(2):
                                    for d in range(9):
                                        di, dj = divmod(d, 3)
                                        nc.tensor.matmul(
                                            pb[:, 0:rows, :],
                                            ws2[ct][:, d, smp,
                                                    ot * P:(ot + 1) * P],
                                            xts[smp][ct][:, y0 + di:
                                                         y0 + di + rows,
                                                         dj:dj + W_SP],
                                            start=first,
                                            stop=(ct == 1 and d == 8))
                                        first = False
                                if eng is nc.vector:
                                    nc.vector.tensor_copy(
                                        ob[:, y0:y0 + rows, :],
                                        pb[:, 0:rows, :])
                                else:
                                    nc.scalar.copy(
                                        ob[:, y0:y0 + rows, :],
                                        pb[:, 0:rows, :])
                                nc.sync.dma_start(
                                    outd[smp, ot, :, y0:y0 + rows, :],
                                    ob[:, y0:y0 + rows, :])
                            continue
                        pb = ps.tile([P, 8, W_SP], f32, name="pb", bufs=2)
                        first = True
                        for ct in range(2):
                            for d in range(9):
                                di, dj = divmod(d, 3)
                                loc = hb * 8 + di
                                nc.tensor.matmul(
                                    pb[:, :, :],
                                    ws2[ct][:, d, smp, ot * P:(ot + 1) * P],
                                    xts[smp][ct][:, loc:loc + 8, dj:dj + W_SP],
                                    start=first, stop=(ct == 1 and d == 8))
                                first = False
                        if hb % 2 == 0:
                            nc.vector.tensor_copy(
                                ob[:, hb * 8:(hb + 1) * 8, :], pb[:, :, :])
                        else:
                            nc.scalar.copy(
                                ob[:, hb * 8:(hb + 1) * 8, :], pb[:, :, :])
                        if last and ot == 1 and hb >= 4:
                            nc.sync.dma_start(
                                outd[smp, ot, :, hb * 8:(hb + 1) * 8, :],
                                ob[:, hb * 8:(hb + 1) * 8, :])
                        elif last and hb % 2 == 1:
                            k = hb // 2
                            nc.sync.dma_start(
                                outd[smp, ot, :, k * 16:(k + 1) * 16, :],
                                ob[:, k * 16:(k + 1) * 16, :])
                    if not last:
                        nc.sync.dma_start(outd[smp, ot, :, :, :], ob[:, :, :])
    nc.finalize()
    return nc


def _host_prep(x, alphas, W, lora_A, lora_B):
    """Host-side layout/dtype transforms (pad/transpose/gather/cast)."""
    import ml_dtypes
    bf16 = ml_dtypes.bfloat16

    xf = np.asarray(x, dtype=np.float32)
    af = np.asarray(alphas, dtype=np.float32)
    Wf = np.asarray(W, dtype=np.float32)
    Af = np.asarray(lora_A, dtype=np.float32).reshape(NR, CIN * K)   # Acat
    Bf = np.asarray(lora_B, dtype=np.float32)

    # padded x, per core: (S, 2, 128, 66, 66) bf16
    xpad = np.zeros((B, CIN, HP, HP), bf16)
    xpad[:, :, 1:-1, 1:-1] = xf.astype(bf16)
    xpad = xpad.reshape(NCORES, S, 2, P, HP, HP)

    # base weights c-major, d-major free layout, doubled along j so the
    # device-side eviction add needs no broadcast: wt[p, d, ct, j, o]
    wth = np.ascontiguousarray(
        Wf.reshape(COUT, CIN, 9).transpose(1, 2, 0)        # [c, d, o]
        .reshape(2, P, 9, COUT)                            # [ct, p, d, o]
        .transpose(1, 2, 0, 3)).astype(bf16)               # [p, d, ct, o]
    wth = np.ascontiguousarray(
        np.broadcast_to(wth[:, :, :, None, :], (P, 9, 2, 2, COUT)))

    # a3[r, d, s, c] = Acat[r, c*9+d-768*s] masked; rows padded 96 -> 128
    a3h = np.zeros((P, 9, 3, CIN), np.float32)
    cc = np.arange(CIN)
    for d in range(9):
        q = cc * 9 + d
        s_of_c = q // (CIN * K)
        q_of_c = q % (CIN * K)
        for s in range(3):
            m = s_of_c == s
            a3h[:NR, d, s, m] = Af[:, q_of_c[m]]
    a3h = a3h[:NR].astype(bf16)

    # b3[r, s, o] = Bcat[3o+s, r];  Bcat = lora_B transposed to [768, 96]
    Bcat = Bf.transpose(1, 0, 2).reshape(COUT * K, NR)
    b3h = np.zeros((P, 3, COUT), np.float32)
    b3h[:NR] = Bcat.reshape(COUT, 3, NR).transpose(2, 1, 0)
    b3h = b3h[:NR].astype(bf16)

    # alph[r, smp] per core (repeat each task 24x; zero rows >= 96).
    # SCALING (alpha/r = 2) folded in here so the device skips one mul.
    alphh = np.zeros((NCORES, NR, S), np.float32)
    rep = np.repeat(af * SCALING, R * K, axis=1)           # [B, 96]
    alphh[:, :, :] = rep.reshape(NCORES, S, NR).transpose(0, 2, 1)

    return xpad, wth, a3h, b3h, alphh


def _in_maps(x, alphas, W, lora_A, lora_B):
    xpad, wth, a3h, b3h, alphh = _host_prep(x, alphas, W, lora_A, lora_B)
    return [
        {"xp": np.ascontiguousarray(xpad[c]), "wt": wth, "a3": a3h, "b3": b3h,
         "alph": np.ascontiguousarray(alphh[c])}
        for c in range(NCORES)
    ]


def kernel(x, alphas, W, lora_A, lora_B):
    from concourse.bass_utils import run_bass_kernel_spmd

    if "nc" not in _CACHE:
        _CACHE["nc"] = _build_nc()
    nc = _CACHE["nc"]

    in_maps = _in_maps(x, alphas, W, lora_A, lora_B)
    res = run_bass_kernel_spmd(nc, in_maps, list(range(NCORES)))
    out = np.empty((B, COUT, H, W_SP), np.float32)
    for c in range(NCORES):
        out[c * S:(c + 1) * S] = res.results[c]["out"].reshape(S, COUT, H, W_SP)
    return out



# revision 44
# speedup vs baseline: 1.0091x; 1.0026x over previous
# MultiLoraConv2d kernel for 8 trn2 NeuronCores (Bass/Tile, data-parallel over batch).
#
# Math (per sample b):
#   delta_flat[b] = sum_t 2*alphas[b,t] * (lora_B[t] @ lora_A[t])        [768, 768]
#   agg[b] = W + delta_flat[b].reshape(COUT, CIN, 3, 3)                  (flat reinterpret)
#   out[b] = conv2d(x[b], agg[b], pad=1)
#
# Device strategy (per core, S = B/8 samples):
#   - All matmul operands bf16 (1 cycle/row on PE, same as fp32r; halves DMA
#     + SBUF). PSUM accumulation stays fp32; max rel err ~2.3e-3 (gate 2e-2).
#   - Host pre-lays-out tensors partition-first; LoRA factors regrouped by
#     (d = 3*i + j, s = (c*9+d)//768) so per-sample aggregated conv weights
#     come out of the PE directly in c-major (stationary) layout:
#       S_d[c, o] = sum_s sum_r a3[d,s,r,c] * (2*alpha_{t(r)} * b3[s,r,o])
#   - Conv = 18 shifted matmuls (9 taps x 2 cin tiles) per PSUM bank;
#     measured issue cadence ~218 ns per 512-col bf16 matmul (hw floor).
#   - Phase 1 does 2 matmuls per bank, not 3: s=(c*9+d)//768 spans only two
#     of the three flat-reshape thirds within any 128-wide c block, so one
#     stationary is all-zero (108 -> 72 matmuls).
#   - Phase-1 eviction (dp + W -> ws2) must keep up with 436ns/bank matmul
#     production: a direct DVE add from PSUM is 691ns (fp32 PSUM operand
#     blocks DVE's 2x mode), so banks are produced into 3-bank PSUM group
#     tiles, Act fuses each group's PSUM->bf16 copy into one ACTIVATE
#     (~1.5us/group), and DVE adds W from the bf16 tmp at the 2x rate
#     (~417ns/bank). GpSimd is useless here: its tensor_scalar is ~3.9us
#     and adds ~1.4us (sw-implemented), and it cannot touch PSUM.
#   - Schedule: 12 dummy warmup matmuls ramp the PE p-state during the ~8us
#     NEFF boot + ~11us first-DMA latency window (wz zeroed on GpSimd so
#     warmup starts right at Tensor boot, not after Act's table load); DMAs
#     in consumption order (alph + full b3 first, a3 chunks, wt interleaved
#     with x prefetches); b3s muls split DVE/Act in consumption order; conv
#     PSUM->SBUF copies alternate DVE/Act; single 8-bank PSUM pool (2x3-bank
#     dp3 groups + 2 pb ring slots, warmup folded into the dp3 tag); output
#     batched 1 DMA per (smp, ot), last sample split per-hb and the final
#     bank split into two 4-row halves to shrink the drain tail.
import numpy as np

B, T, R, ALPHA = 32, 4, 8, 16
CIN, COUT, K = 256, 256, 3
H = W_SP = 64
SCALING = ALPHA / R
NCORES = 8
S = B // NCORES      # samples per core
NR = T * R * K       # 96 lora rows (padded to 128 partitions)
P = 128
HP = H + 2           # 66 padded

_CACHE = {}


def _build_nc():
    import concourse.bacc as bacc
    import concourse.mybir as mybir
    import concourse.tile as tile

    f32 = mybir.dt.float32
    bf16 = mybir.dt.bfloat16

    nc = bacc.Bacc("TRN2", target_bir_lowering=False, debug=False, num_devices=NCORES)

    xp = nc.declare_dram_parameter("xp", [S, 2, P, HP, HP], bf16, isOutput=False)
    wt = nc.declare_dram_parameter("wt", [P, 9, 2, 2, COUT], bf16, isOutput=False)
    a3 = nc.declare_dram_parameter("a3", [NR, 9, 3, CIN], bf16, isOutput=False)
    b3 = nc.declare_dram_parameter("b3", [NR, 3, COUT], bf16, isOutput=False)
    alph = nc.declare_dram_parameter("alph", [NR, S], f32, isOutput=False)
    outd = nc.declare_dram_parameter("out", [S, 2, P, H, W_SP], f32, isOutput=True)

    with tile.TileContext(nc) as tc:
        with tc.tile_pool(name="persist", bufs=1) as persist, \
             tc.tile_pool(name="xt_pool", bufs=6) as xt_pool, \
             tc.tile_pool(name="ob_pool", bufs=2) as ob_pool, \
             tc.tile_pool(name="ps", bufs=1, space="PSUM") as ps:
            a3_sb = persist.tile([P, 9, 3, CIN], bf16)
            b3_sb = persist.tile([P, 3, COUT], bf16)
            alph_sb = persist.tile([P, S], f32)
            wt_sb = persist.tile([P, 9, 2, 2, COUT], bf16)
            b3s = [persist.tile([P, 3, 2, COUT], bf16, name=f"b3s{h}")
                   for h in range(2)]
            ws2 = [persist.tile([P, 9, S, COUT], bf16, name=f"ws2{c}")
                   for c in range(2)]
            wz = persist.tile([P, 704], bf16)
            tmp3 = [persist.tile([P, 3, 2, COUT], bf16, name=f"tmp3{g}")
                    for g in range(4)]

            # PE warmup: p-state ramps to full clock after ~5us of continuous
            # execution; burn that in during NEFF boot / first DMAs so real
            # matmuls run near 2.4GHz from the start (9 was measured too few:
            # the first ~10 real matmuls ran at 427ns). wz is zeroed on
            # GpSimd (boots at ~0.1us) so warmup starts right at Tensor
            # engine boot (~7.2us) instead of waiting for Act's 1.3us
            # ACT_TABLE_LOAD + memzero (~9.5us). The dummy Act copy still
            # pulls the ACT_TABLE_LOAD off the b3s-mul critical path (its
            # target range is outside the warmup APs so it gates nothing).
            nc.gpsimd.memset(wz[:, :], 0.0)
            nc.scalar.copy(wz[:, 672:688], wz[:, 640:656])
            warm = ps.tile([P, 3, 2, COUT], f32, name="dp3", bufs=2)
            for _ in range(12):
                nc.tensor.matmul(warm[:, 0, :, :], wz[:, 0:128], wz[:, 128:640],
                                 start=True, stop=True)

            # DMAs in consumption order on the SP queue (issuing from a
            # second DGE queue halves per-queue DMA bandwidth kernel-wide —
            # measured +60us). alph + full b3 first (gate the b3s muls),
            # a3 next (gates phase-1 matmuls), wt interleaved with the x0
            # prefetches; first DMA completion lands ~11.3us (pipe boot).
            # Fine-grained early chunks: the consolidated matmul waits then
            # gate on small transfers, so phase-1 starts ~12.3us instead of
            # ~14.3 (a 3-d a3 chunk pushes the wait out). Early start only
            # pays now that the eviction pipeline is rate-matched — with
            # all-Act copies it just wrapped the PSUM ring and drooped the
            # p-state (v6: net loss).
            nc.sync.dma_start(alph_sb[0:NR, :], alph[:, :])
            nc.sync.dma_start(b3_sb[0:NR, 0:1], b3[:, 0:1])
            nc.sync.dma_start(b3_sb[0:NR, 1:3], b3[:, 1:3])
            nc.sync.dma_start(a3_sb[0:NR, 0:1], a3[:, 0:1])
            nc.sync.dma_start(a3_sb[0:NR, 1:3], a3[:, 1:3])
            nc.sync.dma_start(wt_sb[:, 0:1], wt[:, 0:1])
            nc.sync.dma_start(a3_sb[0:NR, 3:6], a3[:, 3:6])
            nc.sync.dma_start(wt_sb[:, 1:2], wt[:, 1:2])
            nc.sync.dma_start(wt_sb[:, 2:3], wt[:, 2:3])
            nc.sync.dma_start(a3_sb[0:NR, 6:9], a3[:, 6:9])
            nc.sync.dma_start(wt_sb[:, 3:4], wt[:, 3:4])
            nc.sync.dma_start(wt_sb[:, 4:5], wt[:, 4:5])
            nc.sync.dma_start(wt_sb[:, 5:6], wt[:, 5:6])
            nc.sync.dma_start(wt_sb[:, 6:7], wt[:, 6:7])
            nc.sync.dma_start(wt_sb[:, 7:8], wt[:, 7:8])
            nc.sync.dma_start(wt_sb[:, 8:9], wt[:, 8:9])
            # (wt is host-doubled along a j=2 dim so the phase-1 W-add is a
            # single [P,2,256] tensor_add per (h,d,ct) with no broadcast AP)
            xts = [[None, None] for _ in range(S)]
            for smp in range(3):
                for ct in range(2):
                    t = xt_pool.tile([P, HP, HP], bf16, name="xt")
                    nc.sync.dma_start(t[:, :, :], xp[smp, ct, :, :, :])
                    xts[smp][ct] = t

            # b3s[h][:, s, j, :] = (2*alphas[2h+j]) * b3[s]  (bf16 out; the
            # 2x scaling is folded into the host-side alph values). DVE
            # (278ns/mul) takes all of h0 plus h1's s0 in consumption
            # order; Act (584ns/mul) takes h1's s1/s2, not needed until
            # bank 19 (~8us later). Moving Act's muls to DVE was tried:
            # it un-delays Act's first ring-freeing copy but overloads DVE
            # (muls + all W-adds) and the ring stall just moves there.
            for s, j in ((0, 0), (0, 1), (1, 0), (1, 1), (2, 0), (2, 1)):
                nc.vector.tensor_scalar_mul(b3s[0][0:NR, s, j, :],
                                            b3_sb[0:NR, s, :],
                                            alph_sb[0:NR, j:j + 1])
            for s, j in ((0, 0), (0, 1)):
                nc.vector.tensor_scalar_mul(b3s[1][0:NR, s, j, :],
                                            b3_sb[0:NR, s, :],
                                            alph_sb[0:NR, 2 + j:3 + j])
            for s, j in ((1, 0), (1, 1), (2, 0), (2, 1)):
                nc.scalar.mul(b3s[1][0:NR, s, j, :], b3_sb[0:NR, s, :],
                              alph_sb[0:NR, 2 + j:3 + j])

            # ---- phase 1: aggregated weights via LoRA matmuls ----
            # dp[c, j, o] = sum_s sum_r a3[d,s,r,c] * b3s[h][s,j,o]; then
            # ws2[ct][c, d, 2h+j, o] = dp[c, j, o] + W[c, d, ct, o].
            # s = (c*9+d)//768, so a 128-wide c block spans only TWO of the
            # three s thirds: ct=0 -> s in {0,1}, ct=1 -> s in {1,2}; the
            # third matmul's stationary is all-zero — skip it (108 -> 72).
            # Eviction must keep up with 436ns/bank matmul production.
            # Measured costs: DVE direct add from PSUM 691ns (fp32 PSUM
            # operand blocks the 2x mode); Act fused 3-bank PSUM->bf16 copy
            # 1541ns (513/bank); DVE all-bf16 2x add 417ns; GpSimd all-SBUF
            # add ~1.4us (sw-implemented; its tensor_scalar is 3.9us — do
            # NOT put muls there). No two engines alone cover the 36 banks
            # inside the 15.7us matmul window, so spread over three:
            # 5 groups direct DVE adds, 4 groups Act-copied + DVE adds,
            # 3 groups Act-copied + GpSimd adds -> DVE ~15.4us, Act ~10.8,
            # GpSimd ~12.4. GpSimd takes the LAST h1 groups: it never holds
            # a PSUM slot (the Act copy frees it) and its banks are not
            # read until conv smp2 (~90us), so even 2x slowness is safe.
            ebank = [(h, d, ct)
                     for h in range(2) for d in range(9) for ct in range(2)]
            for g in range(12):
                dpz = ps.tile([P, 3, 2, COUT], f32, name="dp3", bufs=2)
                for i in range(3):
                    h, d, ct = ebank[3 * g + i]
                    s_pair = (0, 1) if ct == 0 else (1, 2)
                    for k, s in enumerate(s_pair):
                        nc.tensor.matmul(
                            dpz[:, i, :, :],
                            a3_sb[0:NR, d, s, ct * P:(ct + 1) * P],
                            b3s[h][0:NR, s, :, :],
                            start=(k == 0), stop=(k == 1))
                # PSUM-free pacing: all-Act copies free banks at 513ns/bank
                # vs 436ns/bank matmul production (ring-wrap stalls, ~3us);
                # moving every 4th group's copy to DVE (~1.8us fused, run in
                # DVE's idle gaps) brings both engines under production
                # rate. tmp ring of 4 keeps the lagging W-adds from gating
                # the copies via WAR reuse.
                t3 = tmp3[g % 4]
                if g % 4 == 3:
                    nc.vector.tensor_copy(t3[:, :, :, :], dpz[:, :, :, :])
                else:
                    nc.scalar.copy(t3[:, :, :, :], dpz[:, :, :, :])
                for i in range(3):
                    h, d, ct = ebank[3 * g + i]
                    nc.vector.tensor_add(
                        ws2[ct][:, d, 2 * h:2 * h + 2, :],
                        t3[:, i, :, :], wt_sb[:, d, ct, :, :])

            # ---- phase 2: per-sample conv, 18 shifted matmuls per psum bank ----
            for smp in range(S):
                if smp == 1:
                    for ct in range(2):
                        t = xt_pool.tile([P, HP, HP], bf16, name="xt")
                        nc.sync.dma_start(t[:, :, :], xp[3, ct, :, :, :])
                        xts[3][ct] = t
                for ot in range(2):
                    ob = ob_pool.tile([P, H, W_SP], f32, name="ob")
                    last = (smp == S - 1)
                    for hb in range(8):
                        if last and ot == 1 and hb == 7:
                            # Final bank split 4+2+2 rows so the drain after
                            # the very last matmul is a 2-row copy + 64KB
                            # DMA; earlier sub-banks' copy/DMA overlap the
                            # later sub-banks' matmuls (~0.1us extra PE
                            # issue overhead, measured -0.36us drain).
                            for y0, rows, eng in ((56, 4, nc.vector),
                                                  (60, 2, nc.scalar),
                                                  (62, 2, nc.vector)):
                                pb = ps.tile([P, 8, W_SP], f32, name="pb",
                                             bufs=2)
                                first = True
                                for ct in range(2):
                                    for d in range(9):
                                        di, dj = divmod(d, 3)
                                        nc.tensor.matmul(
                                            pb[:, 0:rows, :],
                                            ws2[ct][:, d, smp,
                                                    ot * P:(ot + 1) * P],
                                            xts[smp][ct][:, y0 + di:
                                                         y0 + di + rows,
                                                         dj:dj + W_SP],
                                            start=first,
                                            stop=(ct == 1 and d == 8))
                                        first = False
                                if eng is nc.vector:
                                    nc.vector.tensor_copy(
                                        ob[:, y0:y0 + rows, :],
                                        pb[:, 0:rows, :])
                                else:
                                    nc.scalar.copy(
                                        ob[:, y0:y0 + rows, :],
                                        pb[:, 0:rows, :])
                                nc.sync.dma_start(
                                    outd[smp, ot, :, y0:y0 + rows, :],
                                    ob[:, y0:y0 + rows, :])
                            continue
                        pb = ps.tile([P, 8, W_SP], f32, name="pb", bufs=2)
                        first = True
                        for ct in range(2):
                            for d in range(9):
                                di, dj = divmod(d, 3)
                                loc = hb * 8 + di
                                nc.tensor.matmul(
                                    pb[:, :, :],
                                    ws2[ct][:, d, smp, ot * P:(ot + 1) * P],
                                    xts[smp][ct][:, loc:loc + 8, dj:dj + W_SP],
                                    start=first, stop=(ct == 1 and d == 8))
                                first = False
                        if hb % 2 == 0:
                            nc.vector.tensor_copy(
                                ob[:, hb * 8:(hb + 1) * 8, :], pb[:, :, :])
                        else:
                            nc.scalar.copy(
                                ob[:, hb * 8:(hb + 1) * 8, :], pb[:, :, :])
                        if last and ot == 1 and hb >= 4:
                            nc.sync.dma_start(
                                outd[smp, ot, :, hb * 8:(hb + 1) * 8, :],
                                ob[:, hb * 8:(hb + 1) * 8, :])
                        elif last and hb % 2 == 1:
                            k = hb // 2
                            nc.sync.dma_start(
                                outd[smp, ot, :, k * 16:(k + 1) * 16, :],
                                ob[:, k * 16:(k + 1) * 16, :])
                    if not last:
                        nc.sync.dma_start(outd[smp, ot, :, :, :], ob[:, :, :])
    nc.finalize()
    return nc


def _host_prep(x, alphas, W, lora_A, lora_B):
    """Host-side layout/dtype transforms (pad/transpose/gather/cast)."""
    import ml_dtypes
    bf16 = ml_dtypes.bfloat16

    xf = np.asarray(x, dtype=np.float32)
    af = np.asarray(alphas, dtype=np.float32)
    Wf = np.asarray(W, dtype=np.float32)
    Af = np.asarray(lora_A, dtype=np.float32).reshape(NR, CIN * K)   # Acat
    Bf = np.asarray(lora_B, dtype=np.float32)

    # padded x, per core: (S, 2, 128, 66, 66) bf16
    xpad = np.zeros((B, CIN, HP, HP), bf16)
    xpad[:, :, 1:-1, 1:-1] = xf.astype(bf16)
    xpad = xpad.reshape(NCORES, S, 2, P, HP, HP)

    # base weights c-major, d-major free layout, doubled along j so the
    # device-side eviction add needs no broadcast: wt[p, d, ct, j, o]
    wth = np.ascontiguousarray(
        Wf.reshape(COUT, CIN, 9).transpose(1, 2, 0)        # [c, d, o]
        .reshape(2, P, 9, COUT)                            # [ct, p, d, o]
        .transpose(1, 2, 0, 3)).astype(bf16)               # [p, d, ct, o]
    wth = np.ascontiguousarray(
        np.broadcast_to(wth[:, :, :, None, :], (P, 9, 2, 2, COUT)))

    # a3[r, d, s, c] = Acat[r, c*9+d-768*s] masked; rows padded 96 -> 128
    a3h = np.zeros((P, 9, 3, CIN), np.float32)
    cc = np.arange(CIN)
    for d in range(9):
        q = cc * 9 + d
        s_of_c = q // (CIN * K)
        q_of_c = q % (CIN * K)
        for s in range(3):
            m = s_of_c == s
            a3h[:NR, d, s, m] = Af[:, q_of_c[m]]
    a3h = a3h[:NR].astype(bf16)

    # b3[r, s, o] = Bcat[3o+s, r];  Bcat = lora_B transposed to [768, 96]
    Bcat = Bf.transpose(1, 0, 2).reshape(COUT * K, NR)
    b3h = np.zeros((P, 3, COUT), np.float32)
    b3h[:NR] = Bcat.reshape(COUT, 3, NR).transpose(2, 1, 0)
    b3h = b3h[:NR].astype(bf16)

    # alph[r, smp] per core (repeat each task 24x; zero rows >= 96).
    # SCALING (alpha/r = 2) folded in here so the device skips one mul.
    alphh = np.zeros((NCORES, NR, S), np.float32)
    rep = np.repeat(af * SCALING, R * K, axis=1)           # [B, 96]
    alphh[:, :, :] = rep.reshape(NCORES, S, NR).transpose(0, 2, 1)

    return xpad, wth, a3h, b3h, alphh


def _in_maps(x, alphas, W, lora_A, lora_B):
    xpad, wth, a3h, b3h, alphh = _host_prep(x, alphas, W, lora_A, lora_B)
    return [
        {"xp": np.ascontiguousarray(xpad[c]), "wt": wth, "a3": a3h, "b3": b3h,
         "alph": np.ascontiguousarray(alphh[c])}
        for c in range(NCORES)
    ]


def kernel(x, alphas, W, lora_A, lora_B):
    from concourse.bass_utils import run_bass_kernel_spmd

    if "nc" not in _CACHE:
        _CACHE["nc"] = _build_nc()
    nc = _CACHE["nc"]

    in_maps = _in_maps(x, alphas, W, lora_A, lora_B)
    res = run_bass_kernel_spmd(nc, in_maps, list(range(NCORES)))
    out = np.empty((B, COUT, H, W_SP), np.float32)
    for c in range(NCORES):
        out[c * S:(c + 1) * S] = res.results[c]["out"].reshape(S, COUT, H, W_SP)
    return out

